# revision 7
# baseline (speedup 1.0000x reference)
"""BrainGCN Trainium2 kernel V2: 2x GCNConv + 3 FC layers over a 100K-node,
1.6M-edge graph, distributed over 8 NeuronCores.

V2 strategy (replaces the DVE-built one-hot design):
- Symmetric norm w = dinv[src]*dinv[dst] is SEPARABLE: gather tables hold
  dinv-prescaled rows (x' = x*dinv on device; h1' = tanh(...)*dinv in the
  L1 epilogue), and dinv[dst] is folded into the epilogue's ACT scale.
  Self-loops are just extra (d,d) edges (w_self = 1/deg = dinv[d]^2).
- The scatter one-hot S (pure 0/1) is HOST-BUILT and streamed from HBM as
  fp8 (16KB/tile) straight into the PE as matmul rhs -> zero per-tile DVE.
- Edges gathered with gpsimd dma_gather, 1024 idx/call (HW ring limit),
  4 SWDGE queues; idx+S loaded in multi-call slabs.
- dst nodes sharded: core c owns [c*12500, (c+1)*12500), packed into 98
  blocks x 128 slots (vector-LPT balanced). Blocks split in 4 groups;
  L1 epilogues complete group-by-group and 4 chunked AllGathers overlap
  the remaining L1 compute. L2 gathers read the allgathered h1' table,
  bucket g = the g-th allgather chunk.
- Epilogues are slot-major: psum[128slots, F] = acc^T @ W accumulated via
  two 64-row matmul halves + a rank-1 (rdinv x bias) matmul; tanh applies
  dinv[dst] via the ACT per-partition scale. No transposes in L1.
"""

import os
import sys
import types

import numpy as np


def _install_ntff_hook():
    if "antenv.axon_hooks" in sys.modules:
        return
    mod = types.ModuleType("antenv.axon_hooks")
    mod._hook = None
    mod.set_axon_ntff_profile_hook = lambda h: setattr(mod, "_hook", h)
    mod.get_axon_ntff_profile_hook = lambda: mod._hook
    sys.modules["antenv.axon_hooks"] = mod
    try:
        import antenv
        antenv.axon_hooks = mod
        from trn_agent_boot.trn_boot import _ntff_profile_via_ctypes
        mod.set_axon_ntff_profile_hook(
            _ntff_profile_via_ctypes("/opt/axon/libaxon_pjrt.so")
        )
    except Exception:
        pass


_install_ntff_hook()

import ml_dtypes
import concourse.bacc as bacc
import concourse.bass as bass  # noqa: F401
import concourse.mybir as mybir
import concourse.tile as tile
from concourse.bass_utils import run_bass_kernel_spmd

# ---------------------------------------------------------------- constants
N = 100000
D_IN = 128
H1 = 64
NCORES = 8
SHARD = N // NCORES            # 12500
BLKN = 98                      # blocks of 128 slots per core
SLOTS = BLKN * 128             # 12544
NCHUNK = -(-N // 128)          # 782 prescale chunks -> pad to bucket multiple
NB = 4                         # buckets / groups
# L1 buckets over (padded) x rows
CH_PER_B = -(-NCHUNK // NB)    # 196 chunks per bucket
XB1 = CH_PER_B * 128           # 25088 rows per L1 bucket table
NPAD = XB1 * NB                # 100352 padded x rows
# block groups (for chunked allgather); L2 buckets = groups
GBLK = [25, 25, 24, 24]
GSTART_B = [0, 25, 50, 74]
GROWS = [g * 128 for g in GBLK]
GSTART_R = [0, 3200, 6400, 9472]
XB2 = [NCORES * r for r in GROWS]   # L2 bucket table rows (25600/24576)
CHUNK_TILES = 8                # tiles per dma_gather call (HW ring limit 1024)
SUPER = 4                      # gather calls per idx/S slab

LAST_EXEC_TIME_NS = None
LAST_RESULTS = None

f32 = mybir.dt.float32
bf16 = mybir.dt.bfloat16
fp8 = mybir.dt.float8e4
i16 = mybir.dt.int16
np_fp8 = ml_dtypes.float8_e4m3fn
np_bf16 = ml_dtypes.bfloat16


# ------------------------------------------------------------- host planning
def _lpt_assign_vec(loads):
    """Pack SHARD nodes into BLKN blocks x 128 slots balancing load vectors."""
    n = loads.shape[0]
    order = np.argsort(-loads.sum(1), kind="stable")
    block_loads = np.zeros((BLKN, loads.shape[1]), np.float64)
    used = np.zeros(BLKN, np.int64)
    pos = np.empty(n, np.int64)
    for i in order:
        li = loads[i]
        cand = block_loads + li
        score = np.einsum("ij,ij->i", cand, cand)
        score[used >= 128] = np.inf
        b = int(np.argmin(score))
        pos[i] = b * 128 + used[b]
        block_loads[b] += li
        used[b] += 1
    return pos


def _build_stream(loc_rows, seg_id, slot, nseg, T_flat):
    """Scatter edges into the padded (idx, S) streams.

    seg_id: segment index per edge (precomputed to match the device
    emission order); T_flat[nseg]: tiles per segment.
    Returns idx_wrapped [128, P//16] int16 and S8 [128, P] fp8.
    """
    P = 128 * int(T_flat.sum())
    seg_base = np.zeros(nseg + 1, np.int64)
    np.cumsum(128 * T_flat, out=seg_base[1:])

    order = np.lexsort((loc_rows, seg_id))
    skey = seg_id[order]
    counts = np.bincount(skey, minlength=nseg)
    starts = np.zeros(nseg + 1, np.int64)
    np.cumsum(counts, out=starts[1:])
    rank = np.arange(len(order)) - starts[skey]
    dest = seg_base[skey] + rank

    out_rows = np.zeros(P, np.int64)
    out_rows[dest] = loc_rows[order]
    idx_wrapped = np.tile(
        out_rows.astype(np.int16).reshape(-1, 16).T, (8, 1)
    )  # [128, P//16]

    S8 = np.zeros((128, P), np_fp8)
    S8[dest % 128, (dest // 128) * 128 + slot[order]] = np_fp8(1.0)
    return idx_wrapped, S8


def _plan(src, dst):
    deg = (np.bincount(dst, minlength=N) + 1.0).astype(np.float64)
    dinv = (1.0 / np.sqrt(deg)).astype(np.float32)
    rdinv = np.sqrt(deg).astype(np.float32)

    # append self edges
    s_all = np.concatenate([src, np.arange(N, dtype=np.int64)])
    d_all = np.concatenate([dst, np.arange(N, dtype=np.int64)])
    core_of = d_all // SHARD

    # LPT balance: per-node load vector = L1 bucket in-counts + total/2
    b1_all = s_all // XB1
    loads = np.zeros((N, NB + 1), np.float64)
    np.add.at(loads, (d_all, b1_all), 1.0)
    loads[:, NB] = loads[:, :NB].sum(1) * 0.5

    pos_local = np.empty(N, np.int64)
    node_of_pos = np.full((NCORES, SLOTS), -1, np.int64)
    for c in range(NCORES):
        nodes = np.arange(c * SHARD, (c + 1) * SHARD)
        p = _lpt_assign_vec(loads[nodes])
        pos_local[nodes] = p
        node_of_pos[c, p] = nodes

    blk_of_pos = np.arange(SLOTS) // 128
    grp_of_blk = np.repeat(np.arange(NB), GBLK)           # [BLKN]
    grp_of_pos = grp_of_blk[blk_of_pos]                   # [SLOTS]
    gstart_r = np.asarray(GSTART_R, np.int64)
    grows = np.asarray(GROWS, np.int64)

    # L2 table row (within bucket g) for each node
    g_node = grp_of_pos[pos_local]                        # [N]
    core_node = np.arange(N) // SHARD
    r2_node = core_node * grows[g_node] + (pos_local - gstart_r[g_node])

    blk_e = pos_local[d_all] // 128
    slot_e = (pos_local[d_all] % 128).astype(np.int64)
    gd_e = grp_of_blk[blk_e]
    be2_e = g_node[s_all]
    r2_e = r2_node[s_all]

    # tile counts per (bucket, block), maxed over cores
    counts1 = np.zeros((NCORES, NB, BLKN), np.int64)
    counts2 = np.zeros((NCORES, NB, BLKN), np.int64)
    for c in range(NCORES):
        m = core_of == c
        counts1[c] = np.bincount(
            b1_all[m] * BLKN + blk_e[m], minlength=NB * BLKN
        ).reshape(NB, BLKN)
        counts2[c] = np.bincount(
            be2_e[m] * BLKN + blk_e[m], minlength=NB * BLKN
        ).reshape(NB, BLKN)
    T1 = np.ceil(counts1.max(axis=0) / 128).astype(np.int64)   # [NB, BLKN]
    T2 = np.ceil(counts2.max(axis=0) / 128).astype(np.int64)

    # segment orders matching device emission
    # L1: (g, be, b in group g); L2: (be, b)
    seg1_index = np.full((NB, BLKN), -1, np.int64)
    T1_flat = []
    k = 0
    for g in range(NB):
        for be in range(NB):
            for b in range(GSTART_B[g], GSTART_B[g] + GBLK[g]):
                seg1_index[be, b] = k
                T1_flat.append(T1[be, b])
                k += 1
    T1_flat = np.asarray(T1_flat, np.int64)
    nseg1 = k

    seg2_index = np.full((NB, BLKN), -1, np.int64)
    T2_flat = []
    k = 0
    for be in range(NB):
        for b in range(BLKN):
            seg2_index[be, b] = k
            T2_flat.append(T2[be, b])
            k += 1
    T2_flat = np.asarray(T2_flat, np.int64)
    nseg2 = k

    streams = []
    dinv_cols_l = []
    rdinv_row_l = []
    for c in range(NCORES):
        m = core_of == c
        loc1 = s_all[m] - b1_all[m] * XB1
        sid1 = seg1_index[b1_all[m], blk_e[m]]
        idx1, S1 = _build_stream(loc1, sid1, slot_e[m], nseg1, T1_flat)
        loc2 = r2_e[m]
        sid2 = seg2_index[be2_e[m], blk_e[m]]
        idx2, S2 = _build_stream(loc2, sid2, slot_e[m], nseg2, T2_flat)
        streams.append((idx1, S1, idx2, S2))

        dv = np.zeros(SLOTS, np.float32)
        rv = np.zeros(SLOTS, np.float32)
        valid = node_of_pos[c] >= 0
        dv[valid] = dinv[node_of_pos[c][valid]]
        rv[valid] = rdinv[node_of_pos[c][valid]]
        dinv_cols_l.append(np.ascontiguousarray(dv.reshape(BLKN, 128).T))
        rdinv_row_l.append(rv.reshape(1, SLOTS))

    # prescale chunk scales: dinv by natural node order, padded
    dch = np.zeros(NPAD, np.float32)
    dch[:N] = dinv
    dinv_chunks = np.ascontiguousarray(dch.reshape(-1, 128).T)  # [128, NPAD/128]

    return (streams, dinv_cols_l, rdinv_row_l, dinv_chunks, T1, T2,
            node_of_pos)


# ------------------------------------------------------------ device program
def _emit_conv(nc, pools, cfg):
    """Gathers + streamed-S scatter matmuls + per-block acc + epilogues."""
    T = cfg["T"]                  # [NB, BLKN] tiles per (bucket, block)
    DF = cfg["feat"]
    table = cfg["table"]          # fn(be) -> DRAM AP
    idx_dram = cfg["idx"]
    s_dram = cfg["s"]
    tag = cfg["tag"]
    sched = cfg["sched"]          # list of (be, b, t)
    sb, ps_run, accp = pools["sb"], pools["ps_run"], pools["accp"]

    # last bucket per block = the last segment in which block b appears
    last_be = np.full(BLKN, -1, np.int64)
    for be, b, t in sched:
        last_be[b] = be

    acc_tiles = {}
    ntiles = len(sched)
    gi = 0
    cur_ps = None
    while gi < ntiles:
        be0 = sched[gi][0]
        K = 1
        while (K < SUPER * CHUNK_TILES and gi + K < ntiles
               and sched[gi + K][0] == be0):
            K += 1
        idx_sl = sb.tile([128, SUPER * CHUNK_TILES * 8], i16, tag="idx")
        nc.sync.dma_start(idx_sl[:, : K * 8], idx_dram[:, gi * 8:(gi + K) * 8])
        s_sl = sb.tile([128, SUPER * CHUNK_TILES * 128], fp8, tag="s")
        nc.scalar.dma_start(
            s_sl[:, : K * 128], s_dram[:, gi * 128:(gi + K) * 128]
        )
        co = 0
        while co < K:
            k = min(CHUNK_TILES, K - co)
            gat = sb.tile([128, CHUNK_TILES, 128], bf16, tag="gat")
            nc.gpsimd.dma_gather(
                gat[:, :k, :], table(be0), idx_sl[:, (co) * 8:(co + k) * 8],
                k * 128, k * 128, 128,
                queue_num=cfg["_q"] % 4,
            )
            cfg["_q"] += 1
            for tl in range(k):
                be, b, t = sched[gi + co + tl]
                if t == 0:
                    cur_ps = ps_run.tile([DF, 128], f32, tag="runps")
                nc.tensor.matmul(
                    cur_ps[:], gat[:, tl, :DF],
                    s_sl[:, (co + tl) * 128:(co + tl + 1) * 128],
                    start=(t == 0), stop=(t == int(T[be][b]) - 1),
                )
                if t == int(T[be][b]) - 1:
                    if b not in acc_tiles:
                        acc_tiles[b] = accp.tile(
                            [DF, 128], f32, tag=f"acc{b}", name=f"acc{tag}_{b}"
                        )
                        nc.vector.tensor_copy(acc_tiles[b][:], cur_ps[:])
                    else:
                        nc.vector.tensor_add(
                            acc_tiles[b][:], acc_tiles[b][:], cur_ps[:]
                        )
                    if be == last_be[b]:
                        cfg["epilogue"](b, acc_tiles[b])
            co += k
        gi += K


def _build_program(T1, T2, wshapes):
    nc = bacc.Bacc("TRN2", num_swdge_queues=4)

    P1 = 128 * int(T1.sum())
    P2 = 128 * int(T2.sum())

    xg_d = nc.dram_tensor("xg", [NPAD, D_IN], bf16, kind="ExternalInput")
    idx1_d = nc.dram_tensor("idx1", [128, P1 // 16], i16, kind="ExternalInput")
    s1_d = nc.dram_tensor("s1", [128, P1], fp8, kind="ExternalInput")
    idx2_d = nc.dram_tensor("idx2", [128, P2 // 16], i16, kind="ExternalInput")
    s2_d = nc.dram_tensor("s2", [128, P2], fp8, kind="ExternalInput")
    dch_d = nc.dram_tensor("dch", [128, NPAD // 128], f32, kind="ExternalInput")
    ident_d = nc.dram_tensor("ident", [128, 128], f32, kind="ExternalInput")
    dcols_d = nc.dram_tensor("dcols", [128, BLKN], f32, kind="ExternalInput")
    rrow_d = nc.dram_tensor("rrow", [1, SLOTS], f32, kind="ExternalInput")
    wdr = {}
    for name, shp in wshapes.items():
        wdr[name] = nc.dram_tensor(name, list(shp), f32, kind="ExternalInput")
    y_d = nc.dram_tensor("y", [BLKN, 128], f32, kind="ExternalOutput")

    sched1 = []
    for g in range(NB):
        for be in range(NB):
            for b in range(GSTART_B[g], GSTART_B[g] + GBLK[g]):
                for t in range(int(T1[be][b])):
                    sched1.append((be, b, t))
    sched2 = []
    for be in range(NB):
        for b in range(BLKN):
            for t in range(int(T2[be][b])):
                sched2.append((be, b, t))

    with tile.TileContext(nc) as tc:
        with (
            tc.tile_pool(name="cst", bufs=1) as cst,
            tc.tile_pool(name="sb", bufs=4) as sb,
            tc.tile_pool(name="px", bufs=4) as px,
            tc.tile_pool(name="accp", bufs=1) as accp,
            tc.tile_pool(name="hp", bufs=4) as hp,
            tc.tile_pool(name="ps_run", bufs=4, space="PSUM") as ps_run,
            tc.tile_pool(name="ps_epi", bufs=3, space="PSUM") as ps_epi,
            tc.tile_pool(name="dram", bufs=1, space="DRAM") as dram,
        ):
            pools = {"sb": sb, "ps_run": ps_run, "accp": accp}

            wt = {}
            for name in wshapes:
                wt[name] = cst.tile(list(wshapes[name]), f32, name=f"w_{name}")
                nc.sync.dma_start(wt[name][:], wdr[name][:])
            dcols_t = cst.tile([128, BLKN], f32)
            nc.sync.dma_start(dcols_t[:], dcols_d[:])
            rrow_t = cst.tile([1, SLOTS], f32)
            nc.sync.dma_start(rrow_t[:], rrow_d[:])
            dch_t = cst.tile([128, NPAD // 128], f32)
            nc.sync.dma_start(dch_t[:], dch_d[:])
            ident_t = cst.tile([128, 128], f32)
            nc.sync.dma_start(ident_t[:], ident_d[:])
            zero64 = cst.tile([128, H1], bf16)
            nc.vector.memset(zero64[:], 0.0)

            xp_g = [dram.tile([XB1, D_IN], bf16, name=f"xp{g}")
                    for g in range(NB)]
            hsh_g = [dram.tile([GROWS[g], 128], bf16, name=f"hsh{g}")
                     for g in range(NB)]
            h1f_g = [dram.tile([XB2[g], 128], bf16, addr_space="Shared",
                               name=f"h1f{g}")
                     for g in range(NB)]

            # ---- prescale x' = x * dinv into bucket tables
            for c in range(NPAD // 128):
                g = c // CH_PER_B
                lc = c % CH_PER_B
                xc = px.tile([128, D_IN], bf16, tag="px")
                nc.sync.dma_start(xc[:], xg_d[c * 128:(c + 1) * 128, :])
                xs = px.tile([128, D_IN], bf16, tag="pxs")
                nc.vector.tensor_scalar(
                    xs[:], xc[:], dch_t[:, c:c + 1], None,
                    mybir.AluOpType.mult,
                )
                nc.scalar.dma_start(xp_g[g][lc * 128:(lc + 1) * 128, :], xs[:])

            # ---- L1 conv
            def epi1(b, acc_t):
                g = int(np.searchsorted(GSTART_B, b, side="right") - 1)
                eps = ps_epi.tile([128, H1], f32, tag="eps")
                nc.tensor.matmul(eps[:], acc_t[:], wt["cW0"][:],
                                 start=True, stop=False)
                nc.tensor.matmul(eps[:], rrow_t[0:1, b * 128:(b + 1) * 128],
                                 wt["cb0r"][:], start=False, stop=True)
                h1s = hp.tile([128, H1], f32, tag="h1s")
                nc.scalar.activation(
                    h1s[:], eps[:], mybir.ActivationFunctionType.Tanh,
                    scale=dcols_t[:, b:b + 1],
                )
                h1n = hp.tile([128, H1], bf16, tag="h1n")
                nc.vector.tensor_scalar(
                    h1n[:], h1s[:], dcols_t[:, b:b + 1], None,
                    mybir.AluOpType.mult,
                )
                r0 = (b - GSTART_B[g]) * 128
                nc.scalar.dma_start(hsh_g[g][r0:r0 + 128, 0:H1], h1n[:])
                nc.sync.dma_start(hsh_g[g][r0:r0 + 128, H1:128], zero64[:])

            cfg1 = {
                "T": T1, "feat": D_IN, "tag": "1",
                "table": lambda be: xp_g[be][:, :],
                "idx": idx1_d, "s": s1_d, "sched": sched1,
                "epilogue": epi1, "_q": 0,
            }
            _emit_conv(nc, pools, cfg1)

            for g in range(NB):
                nc.gpsimd.collective_compute(
                    "AllGather",
                    mybir.AluOpType.bypass,
                    ins=[hsh_g[g].opt()],
                    outs=[h1f_g[g].opt()],
                    replica_groups=[list(range(NCORES))],
                )

            # ---- L2 conv + FC stack
            def epi2(b, acc_t):
                e1 = ps_epi.tile([128, H1], f32, tag="eps")
                nc.tensor.matmul(e1[:], acc_t[:], wt["cW1"][:],
                                 start=True, stop=False)
                nc.tensor.matmul(e1[:], rrow_t[0:1, b * 128:(b + 1) * 128],
                                 wt["cb1r"][:], start=False, stop=True)
                h2s = hp.tile([128, H1], f32, tag="h2s")
                nc.scalar.activation(
                    h2s[:], e1[:], mybir.ActivationFunctionType.Tanh,
                    scale=dcols_t[:, b:b + 1],
                )
                tp = ps_epi.tile([H1, 128], f32, tag="eps")
                nc.tensor.transpose(tp[:], h2s[:], ident_t[:])
                h2T = hp.tile([H1, 128], f32, tag="h2T")
                nc.vector.tensor_copy(h2T[:], tp[:])
                e2 = ps_epi.tile([H1, 128], f32, tag="eps")
                nc.tensor.matmul(e2[:], wt["fW0"][:], h2T[:],
                                 start=True, stop=True)
                h3T = hp.tile([H1, 128], f32, tag="h3T")
                nc.scalar.activation(
                    h3T[:], e2[:], mybir.ActivationFunctionType.Tanh,
                    bias=wt["fb0"][:, 0:1],
                )
                e3 = ps_epi.tile([32, 128], f32, tag="eps")
                nc.tensor.matmul(e3[:], wt["fW1"][:], h3T[:],
                                 start=True, stop=True)
                h4T = hp.tile([32, 128], f32, tag="h4T")
                nc.scalar.activation(
                    h4T[:], e3[:], mybir.ActivationFunctionType.Tanh,
                    bias=wt["fb1"][:, 0:1],
                )
                e4 = ps_epi.tile([1, 128], f32, tag="eps")
                nc.tensor.matmul(e4[:], wt["fW2"][:], h4T[:],
                                 start=True, stop=True)
                yrow = hp.tile([1, 128], f32, tag="yrow")
                nc.vector.tensor_scalar_add(yrow[:], e4[:], wt["fb2"][0:1, 0:1])
                nc.sync.dma_start(y_d[b:b + 1, :], yrow[:])

            cfg2 = {
                "T": T2, "feat": H1, "tag": "2",
                "table": lambda be: h1f_g[be][:, :],
                "idx": idx2_d, "s": s2_d, "sched": sched2,
                "epilogue": epi2, "_q": cfg1["_q"],
            }
            _emit_conv(nc, pools, cfg2)

    nc.compile()
    return nc


# ------------------------------------------------------------------- driver
def kernel(**inputs):
    global LAST_EXEC_TIME_NS, LAST_RESULTS

    x = np.asarray(inputs["x"], np.float32)
    ei = np.asarray(inputs["edge_index"], np.int64)
    src, dst = ei[0], ei[1]

    weights = {
        "cW0": np.ascontiguousarray(np.asarray(inputs["cW0"], np.float32)),
        "cb0r": np.asarray(inputs["cb0"], np.float32).reshape(1, H1),
        "cW1": np.ascontiguousarray(np.asarray(inputs["cW1"], np.float32)),
        "cb1r": np.asarray(inputs["cb1"], np.float32).reshape(1, H1),
        "fW0": np.ascontiguousarray(np.asarray(inputs["fW0"], np.float32)),
        "fb0": np.asarray(inputs["fb0"], np.float32).reshape(H1, 1),
        "fW1": np.ascontiguousarray(np.asarray(inputs["fW1"], np.float32)),
        "fb1": np.asarray(inputs["fb1"], np.float32).reshape(32, 1),
        "fW2": np.ascontiguousarray(np.asarray(inputs["fW2"], np.float32)),
        "fb2": np.asarray(inputs["fb2"], np.float32).reshape(1, 1),
    }

    (streams, dinv_cols_l, rdinv_row_l, dinv_chunks, T1, T2,
     node_of_pos) = _plan(src, dst)

    nc = _build_program(T1, T2, {k: v.shape for k, v in weights.items()})

    xpad = np.zeros((NPAD, D_IN), np_bf16)
    xpad[:N] = x.astype(np_bf16)

    in_maps = []
    for c in range(NCORES):
        idx1, S1, idx2, S2 = streams[c]
        m = {"xg": xpad, "idx1": idx1, "s1": S1, "idx2": idx2, "s2": S2,
             "dch": dinv_chunks, "dcols": dinv_cols_l[c],
             "rrow": rdinv_row_l[c], "ident": np.eye(128, dtype=np.float32)}
        m.update(weights)
        in_maps.append(m)

    trace = os.environ.get("BASS_GCN_TRACE") == "1"
    res = run_bass_kernel_spmd(nc, in_maps, list(range(NCORES)), trace=trace)
    if trace:
        LAST_EXEC_TIME_NS = res.exec_time_ns
    LAST_RESULTS = res

    out = np.zeros((N, 1), np.float32)
    for c in range(NCORES):
        yflat = res.results[c]["y"].reshape(SLOTS)
        valid = node_of_pos[c] >= 0
        out[node_of_pos[c][valid], 0] = yflat[valid]
    return out


# revision 17
# speedup vs baseline: 1.1088x; 1.1088x over previous
"""BrainGCN Trainium2 kernel V2: 2x GCNConv + 3 FC layers over a 100K-node,
1.6M-edge graph, distributed over 8 NeuronCores.

V2 strategy (replaces the DVE-built one-hot design):
- Symmetric norm w = dinv[src]*dinv[dst] is SEPARABLE: gather tables hold
  dinv-prescaled rows (x' = x*dinv on device; h1' = tanh(...)*dinv in the
  L1 epilogue), and dinv[dst] is folded into the epilogue's ACT scale.
  Self-loops are just extra (d,d) edges (w_self = 1/deg = dinv[d]^2).
- The scatter one-hot S (pure 0/1) is HOST-BUILT and streamed from HBM as
  fp8 (16KB/tile) straight into the PE as matmul rhs -> zero per-tile DVE.
- Edges gathered with gpsimd dma_gather, 1024 idx/call (HW ring limit),
  4 SWDGE queues; idx+S loaded in multi-call slabs.
- dst nodes sharded: core c owns [c*12500, (c+1)*12500), packed into 98
  blocks x 128 slots (vector-LPT balanced). Blocks split in 4 groups;
  L1 epilogues complete group-by-group and 4 chunked AllGathers overlap
  the remaining L1 compute. L2 gathers read the allgathered h1' table,
  bucket g = the g-th allgather chunk.
- Epilogues are slot-major: psum[128slots, F] = acc^T @ W accumulated via
  two 64-row matmul halves + a rank-1 (rdinv x bias) matmul; tanh applies
  dinv[dst] via the ACT per-partition scale. No transposes in L1.
"""

import os
import sys
import types

import numpy as np


def _install_ntff_hook():
    if "antenv.axon_hooks" in sys.modules:
        return
    mod = types.ModuleType("antenv.axon_hooks")
    mod._hook = None
    mod.set_axon_ntff_profile_hook = lambda h: setattr(mod, "_hook", h)
    mod.get_axon_ntff_profile_hook = lambda: mod._hook
    sys.modules["antenv.axon_hooks"] = mod
    try:
        import antenv
        antenv.axon_hooks = mod
        from trn_agent_boot.trn_boot import _ntff_profile_via_ctypes
        mod.set_axon_ntff_profile_hook(
            _ntff_profile_via_ctypes("/opt/axon/libaxon_pjrt.so")
        )
    except Exception:
        pass


_install_ntff_hook()

import ml_dtypes
import concourse.bacc as bacc
import concourse.bass as bass  # noqa: F401
import concourse.mybir as mybir
import concourse.tile as tile
from concourse.bass_utils import run_bass_kernel_spmd

# ---------------------------------------------------------------- constants
N = 100000
D_IN = 128
H1 = 64
NCORES = 8
SHARD = N // NCORES            # 12500
BLKN = 98                      # blocks of 128 slots per core
SLOTS = BLKN * 128             # 12544
NCHUNK = -(-N // 128)          # 782 prescale chunks -> pad to bucket multiple
NB = 4                         # buckets / groups
# L1 buckets over (padded) x rows
CH_PER_B = -(-NCHUNK // NB)    # 196 chunks per bucket
XB1 = CH_PER_B * 128           # 25088 rows per L1 bucket table
NPAD = XB1 * NB                # 100352 padded x rows
# block groups (for chunked allgather); L2 buckets = groups
GBLK = [25, 25, 24, 24]
GSTART_B = [0, 25, 50, 74]
GROWS = [g * 128 for g in GBLK]
GSTART_R = [0, 3200, 6400, 9472]
XB2 = [NCORES * r for r in GROWS]   # L2 bucket table rows (25600/24576)
CHUNK_TILES = 8                # tiles per dma_gather call (HW ring limit 1024)
SUPER = 4                      # gather calls per idx/S slab

LAST_EXEC_TIME_NS = None
LAST_RESULTS = None

f32 = mybir.dt.float32
bf16 = mybir.dt.bfloat16
fp8 = mybir.dt.float8e4
i16 = mybir.dt.int16
np_fp8 = ml_dtypes.float8_e4m3fn
np_bf16 = ml_dtypes.bfloat16


# ------------------------------------------------------------- host planning
def _lpt_assign_vec(loads):
    """Pack SHARD nodes into BLKN blocks x 128 slots balancing load vectors."""
    n = loads.shape[0]
    order = np.argsort(-loads.sum(1), kind="stable")
    block_loads = np.zeros((BLKN, loads.shape[1]), np.float64)
    used = np.zeros(BLKN, np.int64)
    pos = np.empty(n, np.int64)
    for i in order:
        li = loads[i]
        cand = block_loads + li
        score = np.einsum("ij,ij->i", cand, cand)
        score[used >= 128] = np.inf
        b = int(np.argmin(score))
        pos[i] = b * 128 + used[b]
        block_loads[b] += li
        used[b] += 1
    return pos


def _build_stream(loc_rows, seg_id, slot, nseg, T_flat, vals=None):
    """Scatter edges into the padded (idx, S[, dv]) streams.

    seg_id: segment index per edge (precomputed to match the device
    emission order); T_flat[nseg]: tiles per segment.
    Returns idx_wrapped [128, P//16] int16, S8 [128, P] fp8, and (if vals
    given) dv [128, P//128] f32 with vals per tile-slot (0 at pads).
    """
    P = 128 * int(T_flat.sum())
    seg_base = np.zeros(nseg + 1, np.int64)
    np.cumsum(128 * T_flat, out=seg_base[1:])

    order = np.lexsort((loc_rows, seg_id))
    skey = seg_id[order]
    counts = np.bincount(skey, minlength=nseg)
    starts = np.zeros(nseg + 1, np.int64)
    np.cumsum(counts, out=starts[1:])
    rank = np.arange(len(order)) - starts[skey]
    dest = seg_base[skey] + rank

    out_rows = np.zeros(P, np.int64)
    out_rows[dest] = loc_rows[order]
    idx_wrapped = np.tile(
        out_rows.astype(np.int16).reshape(-1, 16).T, (8, 1)
    )  # [128, P//16]

    S8 = np.zeros((128, P), np_fp8)
    S8[dest % 128, (dest // 128) * 128 + slot[order]] = np_fp8(1.0)
    if vals is None:
        return idx_wrapped, S8, None
    dv = np.zeros((128, P // 128), np.float32)
    dv[dest % 128, dest // 128] = vals[order]
    return idx_wrapped, S8, dv


def _plan(src, dst):
    deg = (np.bincount(dst, minlength=N) + 1.0).astype(np.float64)
    dinv = (1.0 / np.sqrt(deg)).astype(np.float32)
    rdinv = np.sqrt(deg).astype(np.float32)

    # append self edges
    s_all = np.concatenate([src, np.arange(N, dtype=np.int64)])
    d_all = np.concatenate([dst, np.arange(N, dtype=np.int64)])
    core_of = d_all // SHARD

    # LPT balance: per-node load vector = L1 bucket in-counts + total/2
    b1_all = s_all // XB1
    loads = np.zeros((N, NB + 1), np.float64)
    np.add.at(loads, (d_all, b1_all), 1.0)
    loads[:, NB] = loads[:, :NB].sum(1) * 0.5

    pos_local = np.empty(N, np.int64)
    node_of_pos = np.full((NCORES, SLOTS), -1, np.int64)
    for c in range(NCORES):
        nodes = np.arange(c * SHARD, (c + 1) * SHARD)
        p = _lpt_assign_vec(loads[nodes])
        pos_local[nodes] = p
        node_of_pos[c, p] = nodes

    blk_of_pos = np.arange(SLOTS) // 128
    grp_of_blk = np.repeat(np.arange(NB), GBLK)           # [BLKN]
    grp_of_pos = grp_of_blk[blk_of_pos]                   # [SLOTS]
    gstart_r = np.asarray(GSTART_R, np.int64)
    grows = np.asarray(GROWS, np.int64)

    # L2 table row (within bucket g) for each node
    g_node = grp_of_pos[pos_local]                        # [N]
    core_node = np.arange(N) // SHARD
    r2_node = core_node * grows[g_node] + (pos_local - gstart_r[g_node])

    blk_e = pos_local[d_all] // 128
    slot_e = (pos_local[d_all] % 128).astype(np.int64)
    gd_e = grp_of_blk[blk_e]
    be2_e = g_node[s_all]
    r2_e = r2_node[s_all]

    # tile counts per (bucket, block), maxed over cores
    counts1 = np.zeros((NCORES, NB, BLKN), np.int64)
    counts2 = np.zeros((NCORES, NB, BLKN), np.int64)
    for c in range(NCORES):
        m = core_of == c
        counts1[c] = np.bincount(
            b1_all[m] * BLKN + blk_e[m], minlength=NB * BLKN
        ).reshape(NB, BLKN)
        counts2[c] = np.bincount(
            be2_e[m] * BLKN + blk_e[m], minlength=NB * BLKN
        ).reshape(NB, BLKN)
    T1 = np.ceil(counts1.max(axis=0) / 128).astype(np.int64)   # [NB, BLKN]
    T2 = np.ceil(counts2.max(axis=0) / 128).astype(np.int64)

    # segment orders matching device emission
    # L1: (g, be, b in group g); L2: (be, b)
    seg1_index = np.full((NB, BLKN), -1, np.int64)
    T1_flat = []
    k = 0
    for g in range(NB):
        for be in range(NB):
            for b in range(GSTART_B[g], GSTART_B[g] + GBLK[g]):
                seg1_index[be, b] = k
                T1_flat.append(T1[be, b])
                k += 1
    T1_flat = np.asarray(T1_flat, np.int64)
    nseg1 = k

    seg2_index = np.full((NB, BLKN), -1, np.int64)
    T2_flat = []
    k = 0
    for be in range(NB):
        for b in range(BLKN):
            seg2_index[be, b] = k
            T2_flat.append(T2[be, b])
            k += 1
    T2_flat = np.asarray(T2_flat, np.int64)
    nseg2 = k

    streams = []
    dinv_cols_l = []
    rdinv_row_l = []
    for c in range(NCORES):
        m = core_of == c
        loc1 = s_all[m] - b1_all[m] * XB1
        sid1 = seg1_index[b1_all[m], blk_e[m]]
        idx1, S1, dv1 = _build_stream(
            loc1, sid1, slot_e[m], nseg1, T1_flat, vals=dinv[s_all[m]]
        )
        loc2 = r2_e[m]
        sid2 = seg2_index[be2_e[m], blk_e[m]]
        idx2, S2, _ = _build_stream(loc2, sid2, slot_e[m], nseg2, T2_flat)
        streams.append((idx1, S1, dv1, idx2, S2))

        dv = np.zeros(SLOTS, np.float32)
        rv = np.zeros(SLOTS, np.float32)
        valid = node_of_pos[c] >= 0
        dv[valid] = dinv[node_of_pos[c][valid]]
        rv[valid] = rdinv[node_of_pos[c][valid]]
        dinv_cols_l.append(np.ascontiguousarray(dv.reshape(BLKN, 128).T))
        rdinv_row_l.append(rv.reshape(1, SLOTS))

    return streams, dinv_cols_l, rdinv_row_l, T1, T2, node_of_pos


# ------------------------------------------------------------ device program
def _emit_conv(nc, pools, cfg):
    """Gathers + streamed-S scatter matmuls + per-block acc + epilogues.

    If cfg["dv"] is set, each gathered tile is scaled by the per-edge
    dinv[src] column (ACT engine) before the scatter matmul.
    If cfg["group_hook"] is set, it is called with g after the last tile
    of tile-group g has been EMITTED (used to interleave the AllGathers
    into the Pool instruction stream).
    """
    T = cfg["T"]                  # [NB, BLKN] tiles per (bucket, block)
    DF = cfg["feat"]
    table = cfg["table"]          # fn(be) -> DRAM AP
    idx_dram = cfg["idx"]
    s_dram = cfg["s"]
    dv_dram = cfg.get("dv")
    tag = cfg["tag"]
    sched = cfg["sched"]          # list of (be, b, t)
    group_end = cfg.get("group_end")   # cumulative tile count per group
    group_hook = cfg.get("group_hook")
    ps_run, accp = pools["ps_run"], pools["accp"]
    gatp, slabp, metap = pools["gatp"], pools["slabp"], pools["metap"]

    # last bucket per block = the last segment in which block b appears
    last_be = np.full(BLKN, -1, np.int64)
    for be, b, t in sched:
        last_be[b] = be

    acc_tiles = {}
    ntiles = len(sched)
    gi = 0
    cur_ps = None
    next_group = 0
    while gi < ntiles:
        be0 = sched[gi][0]
        K = 1
        while (K < SUPER * CHUNK_TILES and gi + K < ntiles
               and sched[gi + K][0] == be0):
            K += 1
        idx_sl = metap.tile([128, SUPER * CHUNK_TILES * 8], i16, tag="idx")
        nc.sync.dma_start(idx_sl[:, : K * 8], idx_dram[:, gi * 8:(gi + K) * 8])
        s_sl = slabp.tile([128, SUPER * CHUNK_TILES * 128], fp8, tag="s")
        nc.scalar.dma_start(
            s_sl[:, : K * 128], s_dram[:, gi * 128:(gi + K) * 128]
        )
        if dv_dram is not None:
            dv_sl = metap.tile([128, SUPER * CHUNK_TILES], f32, tag="dv")
            nc.sync.dma_start(dv_sl[:, :K], dv_dram[:, gi:gi + K])
        co = 0
        while co < K:
            k = min(CHUNK_TILES, K - co)
            gat = gatp.tile([128, CHUNK_TILES, 128], bf16, tag="gat")
            nc.gpsimd.dma_gather(
                gat[:, :k, :], table(be0), idx_sl[:, (co) * 8:(co + k) * 8],
                k * 128, k * 128, 128,
                queue_num=cfg["_q"] % 4,
            )
            cfg["_q"] += 1
            for tl in range(k):
                be, b, t = sched[gi + co + tl]
                if dv_dram is not None:
                    lhs = gatp.tile([128, DF], bf16, tag="gsc")
                    nc.scalar.mul(
                        lhs[:], gat[:, tl, :DF],
                        dv_sl[:, co + tl:co + tl + 1],
                    )
                    lhs_ap = lhs[:]
                else:
                    lhs_ap = gat[:, tl, :DF]
                if t == 0:
                    cur_ps = ps_run.tile([DF, 128], f32, tag="runps")
                nc.tensor.matmul(
                    cur_ps[:], lhs_ap,
                    s_sl[:, (co + tl) * 128:(co + tl + 1) * 128],
                    start=(t == 0), stop=(t == int(T[be][b]) - 1),
                )
                if t == int(T[be][b]) - 1:
                    if b not in acc_tiles:
                        acc_tiles[b] = accp.tile(
                            [DF, 128], f32, tag=f"acc{b}", name=f"acc{tag}_{b}"
                        )
                        nc.vector.tensor_copy(acc_tiles[b][:], cur_ps[:])
                    else:
                        nc.vector.tensor_add(
                            acc_tiles[b][:], acc_tiles[b][:], cur_ps[:]
                        )
                    if be == last_be[b]:
                        cfg["epilogue"](b, acc_tiles[b])
            co += k
        gi += K
        while (group_hook is not None and next_group < NB
               and gi >= group_end[next_group]):
            group_hook(next_group)
            next_group += 1


def _build_program(T1, T2, wshapes):
    nc = bacc.Bacc("TRN2", num_swdge_queues=4)

    P1 = 128 * int(T1.sum())
    P2 = 128 * int(T2.sum())

    xg_d = nc.dram_tensor("xg", [NPAD, D_IN], bf16, kind="ExternalInput")
    idx1_d = nc.dram_tensor("idx1", [128, P1 // 16], i16, kind="ExternalInput")
    s1_d = nc.dram_tensor("s1", [128, P1], fp8, kind="ExternalInput")
    dv1_d = nc.dram_tensor("dv1", [128, P1 // 128], f32, kind="ExternalInput")
    idx2_d = nc.dram_tensor("idx2", [128, P2 // 16], i16, kind="ExternalInput")
    s2_d = nc.dram_tensor("s2", [128, P2], fp8, kind="ExternalInput")
    ident_d = nc.dram_tensor("ident", [128, 128], f32, kind="ExternalInput")
    dcols_d = nc.dram_tensor("dcols", [128, BLKN], f32, kind="ExternalInput")
    rrow_d = nc.dram_tensor("rrow", [1, SLOTS], f32, kind="ExternalInput")
    wdr = {}
    for name, shp in wshapes.items():
        wdr[name] = nc.dram_tensor(name, list(shp), f32, kind="ExternalInput")
    y_d = nc.dram_tensor("y", [BLKN, 128], f32, kind="ExternalOutput")

    sched1 = []
    for g in range(NB):
        for be in range(NB):
            for b in range(GSTART_B[g], GSTART_B[g] + GBLK[g]):
                for t in range(int(T1[be][b])):
                    sched1.append((be, b, t))
    sched2 = []
    for be in range(NB):
        for b in range(BLKN):
            for t in range(int(T2[be][b])):
                sched2.append((be, b, t))

    with tile.TileContext(nc) as tc:
        with (
            tc.tile_pool(name="cst", bufs=1) as cst,
            tc.tile_pool(name="gatp", bufs=6) as gatp,
            tc.tile_pool(name="slabp", bufs=3) as slabp,
            tc.tile_pool(name="metap", bufs=3) as metap,
            tc.tile_pool(name="accp", bufs=1) as accp,
            tc.tile_pool(name="hp", bufs=4) as hp,
            tc.tile_pool(name="ps_run", bufs=4, space="PSUM") as ps_run,
            tc.tile_pool(name="ps_epi", bufs=3, space="PSUM") as ps_epi,
            tc.tile_pool(name="dram", bufs=1, space="DRAM") as dram,
        ):
            pools = {"gatp": gatp, "slabp": slabp, "metap": metap,
                     "ps_run": ps_run, "accp": accp}

            wt = {}
            for name in wshapes:
                wt[name] = cst.tile(list(wshapes[name]), f32, name=f"w_{name}")
                nc.sync.dma_start(wt[name][:], wdr[name][:])
            dcols_t = cst.tile([128, BLKN], f32)
            nc.sync.dma_start(dcols_t[:], dcols_d[:])
            rrow_t = cst.tile([1, SLOTS], f32)
            nc.sync.dma_start(rrow_t[:], rrow_d[:])
            ident_t = cst.tile([128, 128], f32)
            nc.sync.dma_start(ident_t[:], ident_d[:])
            zero64 = cst.tile([128, H1], bf16)
            nc.vector.memset(zero64[:], 0.0)

            hsh_g = [dram.tile([GROWS[g], 128], bf16, name=f"hsh{g}")
                     for g in range(NB)]
            h1f_g = [dram.tile([XB2[g], 128], bf16, addr_space="Shared",
                               name=f"h1f{g}")
                     for g in range(NB)]

            # ---- L1 conv
            def epi1(b, acc_t):
                g = int(np.searchsorted(GSTART_B, b, side="right") - 1)
                eps = ps_epi.tile([128, H1], f32, tag="eps")
                nc.tensor.matmul(eps[:], acc_t[:], wt["cW0"][:],
                                 start=True, stop=False)
                nc.tensor.matmul(eps[:], rrow_t[0:1, b * 128:(b + 1) * 128],
                                 wt["cb0r"][:], start=False, stop=True)
                h1s = hp.tile([128, H1], f32, tag="h1s")
                nc.scalar.activation(
                    h1s[:], eps[:], mybir.ActivationFunctionType.Tanh,
                    scale=dcols_t[:, b:b + 1],
                )
                h1n = hp.tile([128, H1], bf16, tag="h1n")
                nc.vector.tensor_scalar(
                    h1n[:], h1s[:], dcols_t[:, b:b + 1], None,
                    mybir.AluOpType.mult,
                )
                r0 = (b - GSTART_B[g]) * 128
                nc.scalar.dma_start(hsh_g[g][r0:r0 + 128, 0:H1], h1n[:])
                nc.sync.dma_start(hsh_g[g][r0:r0 + 128, H1:128], zero64[:])

            group_end = []
            acc = 0
            for g in range(NB):
                n = sum(int(T1[be][b]) for be in range(NB)
                        for b in range(GSTART_B[g], GSTART_B[g] + GBLK[g]))
                acc += n
                group_end.append(acc)

            def ag_hook(g):
                nc.gpsimd.collective_compute(
                    "AllGather",
                    mybir.AluOpType.bypass,
                    ins=[hsh_g[g].opt()],
                    outs=[h1f_g[g].opt()],
                    replica_groups=[list(range(NCORES))],
                )

            cfg1 = {
                "T": T1, "feat": D_IN, "tag": "1",
                "table": lambda be: xg_d[be * XB1:(be + 1) * XB1, :],
                "idx": idx1_d, "s": s1_d, "dv": dv1_d, "sched": sched1,
                "epilogue": epi1, "_q": 0,
                "group_end": group_end, "group_hook": ag_hook,
            }
            _emit_conv(nc, pools, cfg1)

            # ---- L2 conv + FC stack
            def epi2(b, acc_t):
                e1 = ps_epi.tile([128, H1], f32, tag="eps")
                nc.tensor.matmul(e1[:], acc_t[:], wt["cW1"][:],
                                 start=True, stop=False)
                nc.tensor.matmul(e1[:], rrow_t[0:1, b * 128:(b + 1) * 128],
                                 wt["cb1r"][:], start=False, stop=True)
                h2s = hp.tile([128, H1], f32, tag="h2s")
                nc.scalar.activation(
                    h2s[:], e1[:], mybir.ActivationFunctionType.Tanh,
                    scale=dcols_t[:, b:b + 1],
                )
                tp = ps_epi.tile([H1, 128], f32, tag="eps")
                nc.tensor.transpose(tp[:], h2s[:], ident_t[:])
                h2T = hp.tile([H1, 128], f32, tag="h2T")
                nc.vector.tensor_copy(h2T[:], tp[:])
                e2 = ps_epi.tile([H1, 128], f32, tag="eps")
                nc.tensor.matmul(e2[:], wt["fW0"][:], h2T[:],
                                 start=True, stop=True)
                h3T = hp.tile([H1, 128], f32, tag="h3T")
                nc.scalar.activation(
                    h3T[:], e2[:], mybir.ActivationFunctionType.Tanh,
                    bias=wt["fb0"][:, 0:1],
                )
                e3 = ps_epi.tile([32, 128], f32, tag="eps")
                nc.tensor.matmul(e3[:], wt["fW1"][:], h3T[:],
                                 start=True, stop=True)
                h4T = hp.tile([32, 128], f32, tag="h4T")
                nc.scalar.activation(
                    h4T[:], e3[:], mybir.ActivationFunctionType.Tanh,
                    bias=wt["fb1"][:, 0:1],
                )
                e4 = ps_epi.tile([1, 128], f32, tag="eps")
                nc.tensor.matmul(e4[:], wt["fW2"][:], h4T[:],
                                 start=True, stop=True)
                yrow = hp.tile([1, 128], f32, tag="yrow")
                nc.vector.tensor_scalar_add(yrow[:], e4[:], wt["fb2"][0:1, 0:1])
                nc.sync.dma_start(y_d[b:b + 1, :], yrow[:])

            cfg2 = {
                "T": T2, "feat": H1, "tag": "2",
                "table": lambda be: h1f_g[be][:, :],
                "idx": idx2_d, "s": s2_d, "sched": sched2,
                "epilogue": epi2, "_q": cfg1["_q"],
            }
            _emit_conv(nc, pools, cfg2)

    nc.compile()
    return nc


# ------------------------------------------------------------------- driver
def kernel(**inputs):
    global LAST_EXEC_TIME_NS, LAST_RESULTS

    x = np.asarray(inputs["x"], np.float32)
    ei = np.asarray(inputs["edge_index"], np.int64)
    src, dst = ei[0], ei[1]

    weights = {
        "cW0": np.ascontiguousarray(np.asarray(inputs["cW0"], np.float32)),
        "cb0r": np.asarray(inputs["cb0"], np.float32).reshape(1, H1),
        "cW1": np.ascontiguousarray(np.asarray(inputs["cW1"], np.float32)),
        "cb1r": np.asarray(inputs["cb1"], np.float32).reshape(1, H1),
        "fW0": np.ascontiguousarray(np.asarray(inputs["fW0"], np.float32)),
        "fb0": np.asarray(inputs["fb0"], np.float32).reshape(H1, 1),
        "fW1": np.ascontiguousarray(np.asarray(inputs["fW1"], np.float32)),
        "fb1": np.asarray(inputs["fb1"], np.float32).reshape(32, 1),
        "fW2": np.ascontiguousarray(np.asarray(inputs["fW2"], np.float32)),
        "fb2": np.asarray(inputs["fb2"], np.float32).reshape(1, 1),
    }

    streams, dinv_cols_l, rdinv_row_l, T1, T2, node_of_pos = _plan(src, dst)

    nc = _build_program(T1, T2, {k: v.shape for k, v in weights.items()})

    xpad = np.zeros((NPAD, D_IN), np_bf16)
    xpad[:N] = x.astype(np_bf16)

    in_maps = []
    for c in range(NCORES):
        idx1, S1, dv1, idx2, S2 = streams[c]
        m = {"xg": xpad, "idx1": idx1, "s1": S1, "dv1": dv1,
             "idx2": idx2, "s2": S2, "dcols": dinv_cols_l[c],
             "rrow": rdinv_row_l[c], "ident": np.eye(128, dtype=np.float32)}
        m.update(weights)
        in_maps.append(m)

    trace = os.environ.get("BASS_GCN_TRACE") == "1"
    res = run_bass_kernel_spmd(nc, in_maps, list(range(NCORES)), trace=trace)
    if trace:
        LAST_EXEC_TIME_NS = res.exec_time_ns
    LAST_RESULTS = res

    out = np.zeros((N, 1), np.float32)
    for c in range(NCORES):
        yflat = res.results[c]["y"].reshape(SLOTS)
        valid = node_of_pos[c] >= 0
        out[node_of_pos[c][valid], 0] = yflat[valid]
    return out
